# revision 4
# baseline (speedup 1.0000x reference)
# Trainium2 Bass kernel for EndPointRepr (span endpoint representations).
#
# reference:
#   h = encoded_input @ W + b                    # [B, S, P]
#   res_k[q] = concat(h[qb[q], s_k[q]], h[qb[q], e_k[q]]) * (e_k[q] >= s_k[q])
#
# Sharding: data-parallel over batch. Core c owns batch c; the host routes
# each valid (e >= s) query to its batch's core. Invalid queries are never
# routed; the host-side result buffers start zeroed.
#
# Device pipeline (bf16 data path, fp32 PSUM accumulation):
#   phase 1: X arrives pre-transposed from the host as xT [D, S] bf16, so
#            the PE does no transposes: per 128-row block, 8 k-block
#            matmuls (xT tile stationary, W moving) accumulate h in PSUM;
#            a DVE tensor_add folds in the bias and writes h to SBUF bf16.
#   phase 2: SBUF-source transpose-mode dma_gather pulls endpoint rows of
#            h straight out of SBUF into column-major result tiles
#            [128, 2, QCAP] (partition = feature, free = query slot), which
#            DMA to DRAM as rT [512, QCAP]. The host un-transposes.
# bf16 keeps the PE at 1 cycle/row (fp32 is 4) and halves all DMA traffic;
# rel err ~3e-3 against the fp32 reference, well inside the 2e-2 gate.
import numpy as np

B, S, D, P = 8, 2048, 1024, 256
NQ = 8192
NCORES = 8
KB = D // 128          # contraction k-blocks
MB = S // 128          # h row blocks
QCAP = 640             # per-endpoint gather capacity (multiple of 128)
SCH = 4                # xT load chunks along S for DMA/matmul overlap
SCHW = S // SCH
SW = QCAP // 16        # idx columns per stream-endpoint

_cache = {}


def _build_nc():
    import concourse.bacc as bacc
    import concourse.mybir as mybir
    import concourse.tile as tile

    f32 = mybir.dt.float32
    bf16 = mybir.dt.bfloat16
    nc = bacc.Bacc("TRN2", target_bir_lowering=False, debug=False,
                   num_devices=NCORES)

    xt = nc.dram_tensor("xt", [D, S], bf16, kind="ExternalInput").ap()
    w = nc.dram_tensor("w", [D, P], bf16, kind="ExternalInput").ap()
    bias = nc.dram_tensor("bias", [128, P], f32, kind="ExternalInput").ap()
    idx = nc.dram_tensor("idx", [128, 4 * SW], mybir.dt.int16,
                         kind="ExternalInput").ap()
    cnt = nc.dram_tensor("cnt", [1, 4], mybir.dt.int32,
                         kind="ExternalInput").ap()
    rt1 = nc.dram_tensor("rt1", [2 * P, QCAP], bf16,
                         kind="ExternalOutput").ap()
    rt2 = nc.dram_tensor("rt2", [2 * P, QCAP], bf16,
                         kind="ExternalOutput").ap()

    with tile.TileContext(nc) as tc:
        with (
            tc.tile_pool(name="consts", bufs=1) as consts,
            tc.tile_pool(name="xin", bufs=2) as xt_pool,
            tc.tile_pool(name="gath", bufs=1) as g_pool,
            tc.tile_pool(name="ps", bufs=4, space="PSUM") as ps_pool,
        ):
            w_sb = consts.tile([128, KB, P], bf16)
            nc.scalar.dma_start(w_sb, w.rearrange("(kb k) p -> k kb p", k=128))
            bias_sb = consts.tile([128, P], f32)
            nc.scalar.dma_start(bias_sb, bias)
            idx_sb = consts.tile([128, 4 * SW], mybir.dt.int16)
            nc.scalar.dma_start(idx_sb, idx)
            cnt_sb = consts.tile([1, 4], mybir.dt.int32)
            nc.scalar.dma_start(cnt_sb, cnt)

            h_sb = consts.tile([128, MB, P], bf16, name="h")

            # phase 1: h = xT.T @ W + b, one [128, P] row-block at a time
            xt_view = xt.rearrange("(kb k) s -> k kb s", k=128)
            for c in range(SCH):
                xt_c = xt_pool.tile([128, KB, SCHW], bf16, tag="xt")
                nc.sync.dma_start(xt_c,
                                  xt_view[:, :, c * SCHW:(c + 1) * SCHW])
                for ml in range(SCHW // 128):
                    m = c * (SCHW // 128) + ml
                    h_ps = ps_pool.tile([128, P], f32, tag="hps")
                    for kb in range(KB):
                        nc.tensor.matmul(
                            h_ps, xt_c[:, kb, ml * 128:(ml + 1) * 128],
                            w_sb[:, kb, :],
                            start=(kb == 0), stop=(kb == KB - 1))
                    nc.vector.tensor_add(h_sb[:, m, :], h_ps, bias_sb)

            # phase 2: per stream-endpoint (s1, e1, s2, e2) one SBUF-source
            # transpose gather + one result DMA.
            from contextlib import ExitStack
            regs = ExitStack()
            for st in range(4):
                creg = regs.enter_context(nc.gpsimd.register(f"cnt{st}"))
                nc.gpsimd.reg_load(creg, cnt_sb[0:1, st:st + 1])
                g_sb = g_pool.tile([128, 2, QCAP], bf16, name=f"g{st}")
                nc.gpsimd.dma_gather(
                    g_sb, h_sb[:, :, :], idx_sb[:, st * SW:(st + 1) * SW],
                    num_idxs=QCAP, num_idxs_reg=creg, elem_size=P,
                    transpose=True,
                    sbuf_tokens_per_rank=128,
                    sbuf_free_dim_per_rank=2 * P,
                    sbuf_free_dim_pad_per_rank=0,
                    sbuf_byte_offset=0,
                )
                r = rt1 if st < 2 else rt2
                half = st % 2
                out_view = r.rearrange("(b p) q -> p b q", p=128)
                nc.scalar.dma_start(out_view[:, 2 * half:2 * half + 2, :],
                                    g_sb)
            regs.close()

    nc.compile()
    return nc


def _get_nc():
    if "nc" not in _cache:
        _cache["nc"] = _build_nc()
    return _cache["nc"]


def _numpy_ref(flag, encoded_input, start_ids_1, end_ids_1, query_batch_idx,
               start_ids_2, end_ids_2, W, b):
    h = encoded_input.astype(np.float32) @ W.astype(np.float32) + \
        b.astype(np.float32)
    qb = np.asarray(query_batch_idx).astype(np.int64)

    def span(s, e):
        s = np.asarray(s).astype(np.int64)
        e = np.asarray(e).astype(np.int64)
        rep = np.concatenate([h[qb, s], h[qb, e]], axis=-1)
        return rep * (e >= s)[:, None].astype(rep.dtype)

    return span(start_ids_1, end_ids_1), span(start_ids_2, end_ids_2)


def kernel(flag, encoded_input, start_ids_1, end_ids_1, query_batch_idx,
           start_ids_2, end_ids_2, W, b):
    import ml_dtypes
    from concourse.bass_utils import run_bass_kernel_spmd

    bf16 = ml_dtypes.bfloat16
    x_full = np.asarray(encoded_input, dtype=np.float32)
    w_np = np.asarray(W, dtype=np.float32)
    b_np = np.asarray(b).astype(np.float32)
    qb = np.asarray(query_batch_idx).astype(np.int64)
    s1 = np.asarray(start_ids_1).astype(np.int64)
    e1 = np.asarray(end_ids_1).astype(np.int64)
    s2 = np.asarray(start_ids_2).astype(np.int64)
    e2 = np.asarray(end_ids_2).astype(np.int64)

    in_range = (qb.min() >= 0 and qb.max() < B and
                all(a.min() >= 0 and a.max() < S for a in (s1, e1, s2, e2)))

    in_maps, ids_all = [], []
    try:
        if not in_range or x_full.shape != (B, S, D):
            raise ValueError("shape/range")
        w_bf = np.ascontiguousarray(w_np).astype(bf16)
        bias_rep = np.ascontiguousarray(
            np.broadcast_to(b_np[None, :], (128, P)), dtype=np.float32)
        for bb in range(B):
            sel = qb == bb
            idx_w = np.full((4, QCAP), -1, np.int16)
            cnt_np = np.zeros((1, 4), np.int32)
            ids_pair = []
            for pi, (s, e) in enumerate([(s1, e1), (s2, e2)]):
                ids = np.nonzero(sel & (e >= s))[0]
                if len(ids) > QCAP:
                    raise ValueError("capacity overflow")
                ids_pair.append(ids)
                n = len(ids)
                idx_w[2 * pi, :n] = s[ids]
                idx_w[2 * pi + 1, :n] = e[ids]
                if n == 0:
                    idx_w[2 * pi, 0] = 0
                    idx_w[2 * pi + 1, 0] = 0
                    n = 1
                cnt_np[0, 2 * pi] = n
                cnt_np[0, 2 * pi + 1] = n
            ids_all.append(ids_pair)
            # wrap each stream's slots: slot j -> (partition j%16, col j//16)
            idx_wr = np.concatenate(
                [idx_w[st].reshape(SW, 16).T for st in range(4)], axis=1)
            idx_wr = np.ascontiguousarray(np.tile(idx_wr, (8, 1)))
            in_maps.append({
                "xt": np.ascontiguousarray(x_full[bb].T).astype(bf16),
                "w": w_bf,
                "bias": bias_rep,
                "idx": idx_wr,
                "cnt": cnt_np,
            })
    except ValueError:
        res1, res2 = _numpy_ref(flag, x_full, s1, e1, qb, s2, e2, w_np, b_np)
        return np.asarray(res1, np.float32), np.asarray(res2, np.float32)

    nc = _get_nc()
    out = run_bass_kernel_spmd(nc, in_maps, core_ids=list(range(NCORES)))
    _cache["last_run"] = out

    res1 = np.zeros((NQ, 2 * P), np.float32)
    res2 = np.zeros((NQ, 2 * P), np.float32)
    for bb in range(B):
        ids1, ids2 = ids_all[bb]
        if len(ids1):
            res1[ids1] = out.results[bb]["rt1"][:, :len(ids1)].T \
                .astype(np.float32)
        if len(ids2):
            res2[ids2] = out.results[bb]["rt2"][:, :len(ids2)].T \
                .astype(np.float32)
    return res1, res2
